# revision 12
# baseline (speedup 1.0000x reference)
"""Tensor-parallel (F-sharded) MoE MLP (Llama4 text experts) for 8 Trainium2 cores.

Strategy: shard the intermediate dimension F across the 8 cores. Core c holds,
for ALL 8 experts, gate columns [c*FL,(c+1)*FL) and up columns [F+c*FL, ...) of
gate_up_proj, plus rows [c*FL,(c+1)*FL) of down_proj (FL = F/8 = 256). Every
core processes ALL tokens (sorted by expert id): for each expert segment it
computes h = up * silu(gate) over its 256-wide F slice and a partial
y_c = h @ Wd_slice; the host sums the 8 partials. Segment widths are the global
expert counts - identical on every core - so one uniform SPMD program covers
all cores with zero token padding and perfect load balance.

DMA design: every transfer is contiguous-to-contiguous with one dma_start
(the HWDGE descriptor generator costs ~630ns per instruction and queues want
>=4KB lines). The host pre-packs DRAM layouts to match the SBUF tiles exactly:
  xP  [128, KB1*T]   chunk-packed: chunk (t0,nb) at cols KB1*t0, layout [k][t]
  wgu [E*128, KB1*F2L] per-expert [p][k][f] flat
  wd  [E*128, KB2*H]   per-expert [p][k][h] flat
  yP  [128, NH*T]    chunk-packed partial output, layout [hh][t] per chunk
x loads ride the gpsimd SWDGE queue (first two on sync for a fast start),
weights and y stores ride the SP HWDGE queue, silu plus 3 of 8 PSUM-evict
casts ride the Activation engine, the rest of the casts and the h-mul ride
the DVE. L2 issues its k=0 matmuls for 4 output tiles first so the PE has
work while the second h tile finishes.

bf16 in, fp32 PSUM, bf16 partial out (host accumulates in fp32). The program
is specialized to the expert-count tuple (cached per counts).
"""

import numpy as np
import ml_dtypes

_BF16 = ml_dtypes.bfloat16
_NC = 8           # cores
_T = 8192         # tokens
_H = 1024         # hidden
_F = 2048         # intermediate
_FL = _F // _NC   # 256 local F columns per core
_F2L = 2 * _FL    # 512 = [gate | up] local columns
_NH = _H // 128   # 8 output row tiles
_KB1 = _H // 128  # 8 contraction chunks for x @ Wgu
_KB2 = _FL // 128  # 2 contraction chunks for h @ Wd

_ACT_HH = (1, 4, 7)  # y-cast tiles evicted by the Activation engine

_nc_cache: dict = {}
last_run = None  # BassKernelResults of the most recent kernel() call


def _chunks_of(n, cap=512):
    """Split n columns into near-equal chunks of at most cap."""
    if n <= 0:
        return []
    npc = -(-n // cap)
    base, rem = divmod(n, npc)
    sizes = [base + 1] * rem + [base] * (npc - rem)
    out, off = [], 0
    for s in sizes:
        out.append((off, s))
        off += s
    return out


def _chunk_list(counts):
    """Global chunk list [(expert, t0, nb)]; ramp-up head, small tail."""
    chunks = []
    t0 = 0
    for e in range(_NC):
        n = int(counts[e])
        for off, nb in _chunks_of(n):
            chunks.append((e, t0 + off, nb))
        t0 += n
    # ramp the first chunk up in small steps so compute starts early
    if chunks and chunks[0][2] > 384:
        e, t0, nb = chunks[0]
        head = [128, 128, 256]
        reps, off = [], 0
        for hsz in head:
            if nb - off <= hsz + 128:
                break
            reps.append((e, t0 + off, hsz))
            off += hsz
        reps.append((e, t0 + off, nb - off))
        chunks[0:1] = reps
    if chunks and chunks[-1][2] > 256:
        e, t0, nb = chunks[-1]
        chunks[-1:] = [(e, t0, nb - 128), (e, t0 + nb - 128, 128)]
    return chunks


def _build(counts):
    import concourse.bacc as bacc
    import concourse.mybir as mybir
    from concourse.tile import TileContext

    nc = bacc.Bacc()

    xP = nc.dram_tensor("xP", [128, _KB1 * _T], mybir.dt.bfloat16, kind="ExternalInput")
    wgu = nc.dram_tensor(
        "wgu", [_NC * 128, _KB1 * _F2L], mybir.dt.bfloat16, kind="ExternalInput"
    )
    wd = nc.dram_tensor(
        "wd", [_NC * 128, _KB2 * _H], mybir.dt.bfloat16, kind="ExternalInput"
    )
    yP = nc.dram_tensor("yP", [128, _NH * _T], mybir.dt.bfloat16, kind="ExternalOutput")

    chunks = _chunk_list(counts)
    nch = len(chunks)

    with TileContext(nc) as tc:
        with (
            tc.tile_pool(name="wgu_p", bufs=1) as wgu_p,
            tc.tile_pool(name="wd_p", bufs=1) as wd_p,
            tc.tile_pool(name="x_p", bufs=4) as x_p,
            tc.tile_pool(name="silu_p", bufs=4) as silu_p,
            tc.tile_pool(name="h_p", bufs=3) as h_p,
            tc.tile_pool(name="y_p", bufs=3) as y_p,
            tc.tile_pool(name="ps1", bufs=4, space="PSUM") as ps1_p,
            tc.tile_pool(name="ps2", bufs=4, space="PSUM") as ps2_p,
        ):
            wgu_sb: dict = {}
            wd_sb: dict = {}
            x_sb: dict = {}

            def issue_wgu_a(e):
                if e in wgu_sb:
                    return
                half = _KB1 * _F2L // 2
                ga = wgu_p.tile(
                    [128, half], mybir.dt.bfloat16, name=f"wguA{e}", tag=f"wguA{e}"
                )
                nc.sync.dma_start(out=ga, in_=wgu[e * 128 : (e + 1) * 128, :half])
                wgu_sb[e] = [ga]

            def issue_wgu_b(e, eng=None):
                if len(wgu_sb.get(e, ())) != 1:
                    return
                half = _KB1 * _F2L // 2
                gb = wgu_p.tile(
                    [128, half], mybir.dt.bfloat16, name=f"wguB{e}", tag=f"wguB{e}"
                )
                (eng or nc.sync).dma_start(
                    out=gb, in_=wgu[e * 128 : (e + 1) * 128, half:]
                )
                wgu_sb[e].append(gb)

            def issue_wd(e, eng=None):
                if e in wd_sb:
                    return
                dt_ = wd_p.tile(
                    [128, _KB2 * _H], mybir.dt.bfloat16, name=f"wd{e}", tag=f"wd{e}"
                )
                (eng or nc.sync).dma_start(
                    out=dt_, in_=wd[e * 128 : (e + 1) * 128, :]
                )
                wd_sb[e] = dt_

            def issue_w(e):
                issue_wgu_a(e)
                issue_wgu_b(e)
                issue_wd(e)

            def issue_x(ci, eng):
                e, t0, nb = chunks[ci]
                xt = x_p.tile(
                    [128, _KB1 * 512], mybir.dt.bfloat16, name=f"x{ci}", tag="x"
                )
                eng.dma_start(
                    out=xt[:, : _KB1 * nb],
                    in_=xP[:, _KB1 * t0 : _KB1 * (t0 + nb)],
                )
                x_sb[ci] = xt

            # prologue across three queues: wguA0 on sync, x on the gpsimd
            # SWDGE queue, wguB0/wd0 on the Activation HWDGE queue
            issue_wgu_a(chunks[0][0])
            for ci0 in range(min(3, nch)):
                issue_x(ci0, nc.gpsimd)
            issue_wgu_b(chunks[0][0], nc.scalar)
            issue_wd(chunks[0][0], nc.scalar)
            for ci0 in range(1, min(3, nch)):
                issue_w(chunks[ci0][0])

            for ci in range(nch):
                e, t0, nb = chunks[ci]
                if ci + 3 < nch:
                    issue_x(ci + 3, nc.gpsimd)
                    issue_w(chunks[ci + 3][0])
                xt = x_sb.pop(ci)
                h_tiles = []
                for i in range(2):  # two 128-row f-local tiles
                    ps_g = ps1_p.tile(
                        [128, 512], mybir.dt.float32, name="ps1", tag="ps1"
                    )
                    for k in range(_KB1):
                        nc.tensor.matmul(
                            out=ps_g[:, :nb],
                            lhsT=wgu_sb[e][i][:, k * 128 : (k + 1) * 128],
                            rhs=xt[:, k * nb : (k + 1) * nb],
                            start=(k == 0),
                            stop=(k == _KB1 - 1),
                        )
                    st = silu_p.tile(
                        [128, 512], mybir.dt.bfloat16, name="silu", tag="silu"
                    )
                    nc.scalar.activation(
                        st[:, :nb], ps_g[:, :nb], mybir.ActivationFunctionType.Silu
                    )
                    ps_u = ps1_p.tile(
                        [128, 512], mybir.dt.float32, name="ps1", tag="ps1"
                    )
                    for k in range(_KB1):
                        nc.tensor.matmul(
                            out=ps_u[:, :nb],
                            lhsT=wgu_sb[e][i][
                                :, (_KB1 + k) * 128 : (_KB1 + k + 1) * 128
                            ],
                            rhs=xt[:, k * nb : (k + 1) * nb],
                            start=(k == 0),
                            stop=(k == _KB1 - 1),
                        )
                    ht = h_p.tile(
                        [128, 512], mybir.dt.bfloat16, name=f"h{i}", tag=f"h{i}"
                    )
                    nc.vector.tensor_mul(
                        out=ht[:, :nb], in0=ps_u[:, :nb], in1=st[:, :nb]
                    )
                    h_tiles.append(ht)
                yt = y_p.tile([128, _NH * 512], mybir.dt.bfloat16, name="y", tag="y")
                for g in range(2):  # hh groups of 4; k=0 first to hide ht1 latency
                    pss = []
                    for j in range(4):
                        ps_y = ps2_p.tile(
                            [128, 512], mybir.dt.float32, name="ps2", tag="ps2"
                        )
                        hh = g * 4 + j
                        nc.tensor.matmul(
                            out=ps_y[:, :nb],
                            lhsT=wd_sb[e][:, hh * 128 : (hh + 1) * 128],
                            rhs=h_tiles[0][:, :nb],
                            start=True,
                            stop=False,
                        )
                        pss.append(ps_y)
                    for j in range(4):
                        hh = g * 4 + j
                        nc.tensor.matmul(
                            out=pss[j][:, :nb],
                            lhsT=wd_sb[e][:, _H + hh * 128 : _H + (hh + 1) * 128],
                            rhs=h_tiles[1][:, :nb],
                            start=False,
                            stop=True,
                        )
                        dst = yt[:, hh * nb : (hh + 1) * nb]
                        if hh in _ACT_HH:
                            nc.scalar.activation(
                                dst, pss[j][:, :nb], mybir.ActivationFunctionType.Copy
                            )
                        else:
                            nc.vector.tensor_copy(dst, pss[j][:, :nb])
                yq = nc.scalar if ci >= nch - 2 else nc.sync
                yq.dma_start(
                    out=yP[:, _NH * t0 : _NH * (t0 + nb)], in_=yt[:, : _NH * nb]
                )
    nc.compile()
    return nc


def kernel(hidden_states, local_expert_indices, gate_up_proj, down_proj):
    from concourse.bass_utils import run_bass_kernel_spmd

    x = np.asarray(hidden_states, dtype=np.float32)
    idx = np.asarray(local_expert_indices).astype(np.int64)
    wgu_all = np.asarray(gate_up_proj, dtype=np.float32)
    wd_all = np.asarray(down_proj, dtype=np.float32)

    T, H = x.shape
    E, _, F2 = wgu_all.shape
    F = F2 // 2
    FL = F // _NC
    assert E == _NC and T == _T and H == _H and F == _F

    order = np.argsort(idx, kind="stable")
    counts = np.bincount(idx, minlength=E)

    key = tuple(int(c) for c in counts)
    if key not in _nc_cache:
        _nc_cache[key] = _build(key)
    nc = _nc_cache[key]

    chunks = _chunk_list(counts)

    # x packed: [128, KB1*T], chunk (t0, nb) occupies cols KB1*t0 .. KB1*(t0+nb)
    # laid out [k][t] (so the SBUF tile slice k*nb:(k+1)*nb is the k-th chunk)
    xs = np.asarray(x[order].T, dtype=_BF16)         # [H, T]
    xv = xs.reshape(_KB1, 128, T).transpose(1, 0, 2)  # [128, k, T]
    xP = np.empty((128, _KB1 * T), dtype=_BF16)
    for e_, t0, nb in chunks:
        xP[:, _KB1 * t0 : _KB1 * (t0 + nb)] = xv[:, :, t0 : t0 + nb].reshape(
            128, _KB1 * nb
        )

    wgu_bf = wgu_all.astype(_BF16)  # [E, H, 2F]
    wd_bf = wd_all.astype(_BF16)    # [E, F, H]

    in_maps = []
    for c in range(_NC):
        g = wgu_bf[:, :, c * FL : (c + 1) * FL]
        u = wgu_bf[:, :, F + c * FL : F + (c + 1) * FL]
        # per expert: [pair i][gate|up][k][128f] flat; tile A = pair 0, B = pair 1
        # g,u: [E, H, FL]; H = KB1*128 (k,p); FL = 2 pairs * 128
        gv = g.reshape(E, _KB1, 128, 2, 128)   # e, k, p, i, f
        uv = u.reshape(E, _KB1, 128, 2, 128)
        st = np.stack([gv, uv], axis=4)        # e, k, p, i, {g,u}, f
        wgu_pk = st.transpose(0, 2, 3, 4, 1, 5)  # e, p, i, {g,u}, k, f
        wgu_p = np.ascontiguousarray(wgu_pk).reshape(E * 128, _KB1 * _F2L)
        wd_c = wd_bf[:, c * FL : (c + 1) * FL, :]     # [E, FL, H]
        wd_pk = wd_c.reshape(E, _KB2, 128, H).transpose(0, 2, 1, 3)
        wd_p = np.ascontiguousarray(wd_pk).reshape(E * 128, _KB2 * H)
        in_maps.append({"xP": xP, "wgu": wgu_p, "wd": wd_p})

    res = run_bass_kernel_spmd(nc, in_maps, core_ids=list(range(_NC)))
    global last_run
    last_run = res

    # unpack chunk-packed yP [128, NH*T] and accumulate partials in fp32
    acc = np.zeros((H, T), np.float32)
    for c in range(_NC):
        yp = np.asarray(res.results[c]["yP"], dtype=np.float32)
        for e_, t0, nb in chunks:
            blk = yp[:, _NH * t0 : _NH * (t0 + nb)].reshape(128, _NH, nb)
            acc[:, t0 : t0 + nb] += blk.transpose(1, 0, 2).reshape(H, nb)
    out = np.empty((T, H), np.float32)
    out[order] = acc.T
    return out


# revision 13
# speedup vs baseline: 1.0092x; 1.0092x over previous
"""Tensor-parallel (F-sharded) MoE MLP (Llama4 text experts) for 8 Trainium2 cores.

Strategy: shard the intermediate dimension F across the 8 cores. Core c holds,
for ALL 8 experts, gate columns [c*FL,(c+1)*FL) and up columns [F+c*FL, ...) of
gate_up_proj, plus rows [c*FL,(c+1)*FL) of down_proj (FL = F/8 = 256). Every
core processes ALL tokens (sorted by expert id): for each expert segment it
computes h = up * silu(gate) over its 256-wide F slice and a partial
y_c = h @ Wd_slice; the host sums the 8 partials. Segment widths are the global
expert counts - identical on every core - so one uniform SPMD program covers
all cores with zero token padding and perfect load balance.

DMA design: every transfer is contiguous-to-contiguous with one dma_start
(the HWDGE descriptor generator costs ~630ns per instruction and queues want
>=4KB lines). The host pre-packs DRAM layouts to match the SBUF tiles exactly:
  xP  [128, KB1*T]   chunk-packed: chunk (t0,nb) at cols KB1*t0, layout [k][t]
  wgu [E*128, KB1*F2L] per-expert [p][k][f] flat
  wd  [E*128, KB2*H]   per-expert [p][k][h] flat
  yP  [128, NH*T]    chunk-packed partial output, layout [hh][t] per chunk
x loads ride the gpsimd SWDGE queue (first two on sync for a fast start),
weights and y stores ride the SP HWDGE queue, silu plus 3 of 8 PSUM-evict
casts ride the Activation engine, the rest of the casts and the h-mul ride
the DVE. L2 issues its k=0 matmuls for 4 output tiles first so the PE has
work while the second h tile finishes.

bf16 in, fp32 PSUM, bf16 partial out (host accumulates in fp32). The program
is specialized to the expert-count tuple (cached per counts).
"""

import numpy as np
import ml_dtypes

_BF16 = ml_dtypes.bfloat16
_NC = 8           # cores
_T = 8192         # tokens
_H = 1024         # hidden
_F = 2048         # intermediate
_FL = _F // _NC   # 256 local F columns per core
_F2L = 2 * _FL    # 512 = [gate | up] local columns
_NH = _H // 128   # 8 output row tiles
_KB1 = _H // 128  # 8 contraction chunks for x @ Wgu
_KB2 = _FL // 128  # 2 contraction chunks for h @ Wd

_ACT_HH = (1, 4, 7)  # y-cast tiles evicted by the Activation engine

_nc_cache: dict = {}
last_run = None  # BassKernelResults of the most recent kernel() call


def _chunks_of(n, cap=512):
    """Split n columns into near-equal chunks of at most cap."""
    if n <= 0:
        return []
    npc = -(-n // cap)
    base, rem = divmod(n, npc)
    sizes = [base + 1] * rem + [base] * (npc - rem)
    out, off = [], 0
    for s in sizes:
        out.append((off, s))
        off += s
    return out


def _chunk_list(counts):
    """Global chunk list [(expert, t0, nb)]; ramp-up head, small tail."""
    chunks = []
    t0 = 0
    for e in range(_NC):
        n = int(counts[e])
        for off, nb in _chunks_of(n):
            chunks.append((e, t0 + off, nb))
        t0 += n
    # ramp the first chunk up in small steps so compute starts early
    if chunks and chunks[0][2] > 384:
        e, t0, nb = chunks[0]
        head = [128, 128, 256]
        reps, off = [], 0
        for hsz in head:
            if nb - off <= hsz + 128:
                break
            reps.append((e, t0 + off, hsz))
            off += hsz
        reps.append((e, t0 + off, nb - off))
        chunks[0:1] = reps
    if chunks and chunks[-1][2] > 256:
        e, t0, nb = chunks[-1]
        chunks[-1:] = [(e, t0, nb - 128), (e, t0 + nb - 128, 128)]
    return chunks


def _build(counts):
    import concourse.bacc as bacc
    import concourse.mybir as mybir
    from concourse.tile import TileContext

    nc = bacc.Bacc()

    xP = nc.dram_tensor("xP", [128, _KB1 * _T], mybir.dt.bfloat16, kind="ExternalInput")
    wgu = nc.dram_tensor(
        "wgu", [_NC * 128, _KB1 * _F2L], mybir.dt.bfloat16, kind="ExternalInput"
    )
    wd = nc.dram_tensor(
        "wd", [_NC * 128, _KB2 * _H], mybir.dt.bfloat16, kind="ExternalInput"
    )
    yP = nc.dram_tensor("yP", [128, _NH * _T], mybir.dt.bfloat16, kind="ExternalOutput")

    chunks = _chunk_list(counts)
    nch = len(chunks)

    with TileContext(nc) as tc:
        with (
            tc.tile_pool(name="wgu_p", bufs=1) as wgu_p,
            tc.tile_pool(name="wd_p", bufs=1) as wd_p,
            tc.tile_pool(name="x_p", bufs=4) as x_p,
            tc.tile_pool(name="silu_p", bufs=4) as silu_p,
            tc.tile_pool(name="h_p", bufs=3) as h_p,
            tc.tile_pool(name="y_p", bufs=3) as y_p,
            tc.tile_pool(name="ps1", bufs=4, space="PSUM") as ps1_p,
            tc.tile_pool(name="ps2", bufs=4, space="PSUM") as ps2_p,
        ):
            wgu_sb: dict = {}
            wd_sb: dict = {}
            x_sb: dict = {}

            def issue_wgu_a(e):
                if e in wgu_sb:
                    return
                half = _KB1 * _F2L // 2
                ga = wgu_p.tile(
                    [128, half], mybir.dt.bfloat16, name=f"wguA{e}", tag=f"wguA{e}"
                )
                nc.sync.dma_start(out=ga, in_=wgu[e * 128 : (e + 1) * 128, :half])
                wgu_sb[e] = [ga]

            def issue_wgu_b(e, eng=None):
                if len(wgu_sb.get(e, ())) != 1:
                    return
                half = _KB1 * _F2L // 2
                gb = wgu_p.tile(
                    [128, half], mybir.dt.bfloat16, name=f"wguB{e}", tag=f"wguB{e}"
                )
                (eng or nc.sync).dma_start(
                    out=gb, in_=wgu[e * 128 : (e + 1) * 128, half:]
                )
                wgu_sb[e].append(gb)

            def issue_wd(e, eng=None):
                if e in wd_sb:
                    return
                dt_ = wd_p.tile(
                    [128, _KB2 * _H], mybir.dt.bfloat16, name=f"wd{e}", tag=f"wd{e}"
                )
                (eng or nc.sync).dma_start(
                    out=dt_, in_=wd[e * 128 : (e + 1) * 128, :]
                )
                wd_sb[e] = dt_

            def issue_w(e):
                issue_wgu_a(e)
                issue_wgu_b(e)
                issue_wd(e)

            def issue_x(ci, eng):
                e, t0, nb = chunks[ci]
                xt = x_p.tile(
                    [128, _KB1 * 512], mybir.dt.bfloat16, name=f"x{ci}", tag="x"
                )
                eng.dma_start(
                    out=xt[:, : _KB1 * nb],
                    in_=xP[:, _KB1 * t0 : _KB1 * (t0 + nb)],
                )
                x_sb[ci] = xt

            # prologue, all on the sync HWDGE queue (fastest cold-start),
            # ordered by first use: x0, gate/up of pair 0, x1, pair 1, x2, wd
            issue_x(0, nc.sync)
            issue_wgu_a(chunks[0][0])
            if nch > 1:
                issue_x(1, nc.sync)
            issue_wgu_b(chunks[0][0])
            if nch > 2:
                issue_x(2, nc.sync)
            issue_wd(chunks[0][0])
            for ci0 in range(1, min(3, nch)):
                issue_w(chunks[ci0][0])

            for ci in range(nch):
                e, t0, nb = chunks[ci]
                if ci + 3 < nch:
                    issue_x(ci + 3, nc.gpsimd)
                    issue_w(chunks[ci + 3][0])
                xt = x_sb.pop(ci)
                h_tiles = []
                for i in range(2):  # two 128-row f-local tiles
                    ps_g = ps1_p.tile(
                        [128, 512], mybir.dt.float32, name="ps1", tag="ps1"
                    )
                    for k in range(_KB1):
                        nc.tensor.matmul(
                            out=ps_g[:, :nb],
                            lhsT=wgu_sb[e][i][:, k * 128 : (k + 1) * 128],
                            rhs=xt[:, k * nb : (k + 1) * nb],
                            start=(k == 0),
                            stop=(k == _KB1 - 1),
                        )
                    st = silu_p.tile(
                        [128, 512], mybir.dt.bfloat16, name="silu", tag="silu"
                    )
                    nc.scalar.activation(
                        st[:, :nb], ps_g[:, :nb], mybir.ActivationFunctionType.Silu
                    )
                    ps_u = ps1_p.tile(
                        [128, 512], mybir.dt.float32, name="ps1", tag="ps1"
                    )
                    for k in range(_KB1):
                        nc.tensor.matmul(
                            out=ps_u[:, :nb],
                            lhsT=wgu_sb[e][i][
                                :, (_KB1 + k) * 128 : (_KB1 + k + 1) * 128
                            ],
                            rhs=xt[:, k * nb : (k + 1) * nb],
                            start=(k == 0),
                            stop=(k == _KB1 - 1),
                        )
                    ht = h_p.tile(
                        [128, 512], mybir.dt.bfloat16, name=f"h{i}", tag=f"h{i}"
                    )
                    nc.vector.tensor_mul(
                        out=ht[:, :nb], in0=ps_u[:, :nb], in1=st[:, :nb]
                    )
                    h_tiles.append(ht)
                yt = y_p.tile([128, _NH * 512], mybir.dt.bfloat16, name="y", tag="y")
                for g in range(2):  # hh groups of 4; k=0 first to hide ht1 latency
                    pss = []
                    for j in range(4):
                        ps_y = ps2_p.tile(
                            [128, 512], mybir.dt.float32, name="ps2", tag="ps2"
                        )
                        hh = g * 4 + j
                        nc.tensor.matmul(
                            out=ps_y[:, :nb],
                            lhsT=wd_sb[e][:, hh * 128 : (hh + 1) * 128],
                            rhs=h_tiles[0][:, :nb],
                            start=True,
                            stop=False,
                        )
                        pss.append(ps_y)
                    for j in range(4):
                        hh = g * 4 + j
                        nc.tensor.matmul(
                            out=pss[j][:, :nb],
                            lhsT=wd_sb[e][:, _H + hh * 128 : _H + (hh + 1) * 128],
                            rhs=h_tiles[1][:, :nb],
                            start=False,
                            stop=True,
                        )
                        dst = yt[:, hh * nb : (hh + 1) * nb]
                        if hh in _ACT_HH:
                            nc.scalar.activation(
                                dst, pss[j][:, :nb], mybir.ActivationFunctionType.Copy
                            )
                        else:
                            nc.vector.tensor_copy(dst, pss[j][:, :nb])
                nc.sync.dma_start(
                    out=yP[:, _NH * t0 : _NH * (t0 + nb)], in_=yt[:, : _NH * nb]
                )
    nc.compile()
    return nc


def kernel(hidden_states, local_expert_indices, gate_up_proj, down_proj):
    from concourse.bass_utils import run_bass_kernel_spmd

    x = np.asarray(hidden_states, dtype=np.float32)
    idx = np.asarray(local_expert_indices).astype(np.int64)
    wgu_all = np.asarray(gate_up_proj, dtype=np.float32)
    wd_all = np.asarray(down_proj, dtype=np.float32)

    T, H = x.shape
    E, _, F2 = wgu_all.shape
    F = F2 // 2
    FL = F // _NC
    assert E == _NC and T == _T and H == _H and F == _F

    order = np.argsort(idx, kind="stable")
    counts = np.bincount(idx, minlength=E)

    key = tuple(int(c) for c in counts)
    if key not in _nc_cache:
        _nc_cache[key] = _build(key)
    nc = _nc_cache[key]

    chunks = _chunk_list(counts)

    # x packed: [128, KB1*T], chunk (t0, nb) occupies cols KB1*t0 .. KB1*(t0+nb)
    # laid out [k][t] (so the SBUF tile slice k*nb:(k+1)*nb is the k-th chunk)
    xs = np.asarray(x[order].T, dtype=_BF16)         # [H, T]
    xv = xs.reshape(_KB1, 128, T).transpose(1, 0, 2)  # [128, k, T]
    xP = np.empty((128, _KB1 * T), dtype=_BF16)
    for e_, t0, nb in chunks:
        xP[:, _KB1 * t0 : _KB1 * (t0 + nb)] = xv[:, :, t0 : t0 + nb].reshape(
            128, _KB1 * nb
        )

    wgu_bf = wgu_all.astype(_BF16)  # [E, H, 2F]
    wd_bf = wd_all.astype(_BF16)    # [E, F, H]

    in_maps = []
    for c in range(_NC):
        g = wgu_bf[:, :, c * FL : (c + 1) * FL]
        u = wgu_bf[:, :, F + c * FL : F + (c + 1) * FL]
        # per expert: [pair i][gate|up][k][128f] flat; tile A = pair 0, B = pair 1
        # g,u: [E, H, FL]; H = KB1*128 (k,p); FL = 2 pairs * 128
        gv = g.reshape(E, _KB1, 128, 2, 128)   # e, k, p, i, f
        uv = u.reshape(E, _KB1, 128, 2, 128)
        st = np.stack([gv, uv], axis=4)        # e, k, p, i, {g,u}, f
        wgu_pk = st.transpose(0, 2, 3, 4, 1, 5)  # e, p, i, {g,u}, k, f
        wgu_p = np.ascontiguousarray(wgu_pk).reshape(E * 128, _KB1 * _F2L)
        wd_c = wd_bf[:, c * FL : (c + 1) * FL, :]     # [E, FL, H]
        wd_pk = wd_c.reshape(E, _KB2, 128, H).transpose(0, 2, 1, 3)
        wd_p = np.ascontiguousarray(wd_pk).reshape(E * 128, _KB2 * H)
        in_maps.append({"xP": xP, "wgu": wgu_p, "wd": wd_p})

    res = run_bass_kernel_spmd(nc, in_maps, core_ids=list(range(_NC)))
    global last_run
    last_run = res

    # unpack chunk-packed yP [128, NH*T] and accumulate partials in fp32
    acc = np.zeros((H, T), np.float32)
    for c in range(_NC):
        yp = np.asarray(res.results[c]["yP"], dtype=np.float32)
        for e_, t0, nb in chunks:
            blk = yp[:, _NH * t0 : _NH * (t0 + nb)].reshape(128, _NH, nb)
            acc[:, t0 : t0 + nb] += blk.transpose(1, 0, 2).reshape(H, nb)
    out = np.empty((T, H), np.float32)
    out[order] = acc.T
    return out
